# revision 3
# baseline (speedup 1.0000x reference)
"""Local Gaussian refinement kernel for Trainium2 (8 NeuronCores, SPMD) — v2.

For each (b, k): round+clip the coarse coordinate, gather the 5x5 patch of
the heatmap around it, masked softmax over the 25 logits, return the
softmax-weighted expected (x, y).

v2 structure (vs the baseline):
  - 273 pair slots laid out as [91 partitions x 3 chunks] (272 real + 1 pad),
    so each of the three span gathers covers every pair chunk in one shot.
  - Three indirect DMAs (the HW SWDGE unroll consumes exactly one index per
    destination partition row) each fetch a contiguous 1032-element span
    (4 heat rows + tail) per pair; the 5x5 window sits at static strides
    i*W+j inside.  Window pad columns j=5..7 are rejected by the same
    distance mask that handles image-edge clipping.  The heat DRAM tensor
    is padded by 5 rows so spans of the pad pair cannot go OOB.
  - Index math is 6 DVE ops on the critical path (round-to-nearest-even via
    +-2^23 with the -2 window offset folded into the subtract); masks and
    weight tables are built in the shadow of the gather.
  - exp() runs directly on the raw gathered spans (Act engine, table
    preloaded via a dummy activation); masking is a 0/1 multiply after,
    so the pre-exp mask add leaves the critical path.
  - The result store is a dma_scatter_add prepared (descgen) during the
    gather wait and triggered after the result tile is written: the tail
    costs trigger+transfer+sem instead of a full HWDGE DMA latency.
    Output rows are 64-float-strided (256B scatter stride requirement);
    host reads cols 0:6 of rows 0:91.
"""

import sys

sys.path.insert(0, "/opt/trn_rl_repo")

import numpy as np

import concourse.bass as bass
import concourse.bacc as bacc
import concourse.tile as tile
from concourse import mybir
from concourse.bass_utils import run_bass_kernel_spmd

# Problem constants (hardcoded per contract).
B, K, H, W = 128, 17, 192, 256
NCORES = 8
BS = B // NCORES  # 16 batches per core
PAIRS = BS * K  # 272 (b,k) pairs per core
PP = 91  # partitions used for pair math
TT = 3  # chunks: pair g = p + 91*t, g in [0, 273)
SLOTS = PP * TT  # 273
HW_ = H * W
R = BS * K * H  # 52224 heatmap rows per core
WN = 5  # window size
RUNW = 8  # gathered elements per window row (5 valid + 3 pad)
NRUN = 16  # runs per partition (3*5 windows rows + 1 dummy)
BIGF = float(2**23)
F32 = mybir.dt.float32
I32 = mybir.dt.int32
I16 = mybir.dt.int16
A = mybir.AluOpType


def _v(ap, offset, dims):
    """Strided view of a tile AP: dims = [[stride, count], ...] (free dims)."""
    return bass.AP(ap.tensor, ap.offset + offset, [ap.ap[0]] + dims)


def build_program(debug=False):
    nc = bacc.Bacc(None, target_bir_lowering=False)
    heat = nc.dram_tensor("heat", [R + 5, W], F32, kind="ExternalInput")
    coords = nc.dram_tensor("coords", [PP, 2 * TT], F32, kind="ExternalInput")
    out = nc.dram_tensor("out", [128, 64], F32, kind="ExternalOutput")
    if debug:
        d_idx = nc.dram_tensor("d_idx", [PP, NRUN], I32, kind="ExternalOutput")
        d_blk = nc.dram_tensor("d_blk", [PP, NRUN * RUNW], F32, kind="ExternalOutput")
        d_mask = nc.dram_tensor("d_mask", [PP, 120], F32, kind="ExternalOutput")
        d_red = nc.dram_tensor("d_red", [PP, 9], F32, kind="ExternalOutput")
        d_base = nc.dram_tensor("d_base", [PP, 6], F32, kind="ExternalOutput")

    dma_sem = nc.alloc_semaphore("store_dma_sem")
    res_sem = nc.alloc_semaphore("res_sem")

    # res lives OUTSIDE the tile pools: Tile must not see the store-prep's
    # read of it, or the scheduler orders the prep after the result write
    # (a no-sync edge) and the descgen lands on the critical path.  Ordering
    # is done manually: res_sem gates the trigger.
    res = nc.alloc_sbuf_tensor("res_sb", [128, 64], F32)

    with tile.TileContext(nc) as tc:
        with tc.tile_pool(name="sb", bufs=1) as pool:
            # ---- coords DMA first: everything hangs off it -------------
            crd = pool.tile([PP, 2 * TT], F32)  # [p, (t,c)]
            nc.sync.dma_start(out=crd[:], in_=coords[:, :])

            # tiny dummy exp: forces the activation-table load to happen
            # here (Act idle) instead of right before the real exp, where
            # Bacc's multi-wait splitting would serialize the 1283ns load
            # onto the critical path.
            dumm = pool.tile([1, 1], F32)
            nc.vector.memset(dumm[:], 0.0)
            nc.scalar.activation(
                dumm[:], dumm[:], mybir.ActivationFunctionType.Exp
            )

            # ---- static tables (Pool iotas + DVE casts/memsets) --------
            # goff[p, t] = g*H*W with g = p + 91*t (f32 exact: < 2^24)
            gi_i = pool.tile([PP, TT], I32)
            nc.gpsimd.iota(gi_i[:], [[PP, TT]], base=0, channel_multiplier=1)
            goff = pool.tile([PP, TT], F32)
            nc.vector.tensor_copy(goff[:], gi_i[:])
            nc.vector.tensor_scalar(goff[:], goff[:], float(HW_), None, A.mult)

            jj_i = pool.tile([PP, TT * RUNW], I32)  # value j over (t, j8)
            nc.gpsimd.iota(jj_i[:], [[0, TT], [1, RUNW]], base=0, channel_multiplier=0)
            jj = pool.tile([PP, TT * RUNW], F32)
            nc.vector.tensor_copy(jj[:], jj_i[:])
            # jm: j for j<5, j+95 for j>=5 — pad columns must never pass the
            # |col-px|<=2 test (a high-side clip puts px within 2 of col
            # cbase+5/6, so plain j would leak wrapped elements in).
            jm = pool.tile([PP, TT * RUNW], F32)
            nc.vector.tensor_scalar(jm[:], jj[:], 4.5, None, A.is_le)
            nc.vector.tensor_scalar(jm[:], jm[:], 1.0, 95.0, A.subtract, A.mult)
            nc.vector.tensor_sub(jm[:], jj[:], jm[:])

            ii_i = pool.tile([PP, TT * WN], I32)  # value i over (t, i)
            nc.gpsimd.iota(ii_i[:], [[0, TT], [1, WN]], base=0, channel_multiplier=0)
            ii = pool.tile([PP, TT * WN], F32)
            nc.vector.tensor_copy(ii[:], ii_i[:])

            # scatter store indices: identity 0..127 (int16), tokens stripe
            # as idx[p, s] = p + 16*s read from partitions 0:16
            sidx_i = pool.tile([128, 8], I32)
            nc.gpsimd.iota(sidx_i[:], [[16, 8]], base=0, channel_multiplier=1)
            nc.vector.tensor_scalar(sidx_i[:], sidx_i[:], 127, None, A.min)
            sidx = pool.tile([128, 8], I16)
            nc.vector.tensor_copy(sidx[:], sidx_i[:])

            # clip bounds, interleaved (t,c): x->W-5 / y->H-5 ; x->W-1 / y->H-1
            bnd5 = pool.tile([PP, 2 * TT], F32)
            nc.vector.memset(_v(bnd5[:], 0, [[2, TT]]), float(W - WN))
            nc.vector.memset(_v(bnd5[:], 1, [[2, TT]]), float(H - WN))
            bnd1 = pool.tile([PP, 2 * TT], F32)
            nc.vector.memset(_v(bnd1[:], 0, [[2, TT]]), float(W - 1))
            nc.vector.memset(_v(bnd1[:], 1, [[2, TT]]), float(H - 1))

            idxi = pool.tile([PP, TT], I32)

            nc.vector.memset(res[:], 0)

            PITCH = 4 * W + RUNW  # 1032-elem span holds the 5x8 window
            blk = pool.tile([PP, TT * PITCH], F32)  # three spans per partition

            # ---- index chain (critical path, 6 DVE ops) ----------------
            with tc.high_priority():
                pxy = pool.tile([PP, 2 * TT], F32)  # 2^23 + round(x)
                nc.vector.tensor_scalar(pxy[:], crd[:], BIGF, None, A.add)
                basef = pool.tile([PP, 2 * TT], F32)  # clip(round(x)-2, 0, hi)
                nc.vector.tensor_scalar(
                    basef[:], pxy[:], BIGF + 2.0, 0.0, A.subtract, A.max
                )
                nc.vector.tensor_tensor(basef[:], basef[:], bnd5[:], A.min)
                idxb = pool.tile([PP, TT], F32)  # ry0*W + cbase
                nc.vector.scalar_tensor_tensor(
                    idxb[:],
                    _v(basef[:], 1, [[2, TT]]),  # ry0
                    float(W),
                    _v(basef[:], 0, [[2, TT]]),  # cbase
                    op0=A.mult,
                    op1=A.add,
                )
                nc.vector.tensor_tensor(idxb[:], idxb[:], goff[:], A.add)
                nc.vector.tensor_copy(idxi[:], idxb[:])

            # ---- store prep BEFORE the gather: Pool executes it while the
            # gather descgen still waits on the index chain.  Tile defers the
            # RAW edge on `res` to the trigger_dma below; the prep only
            # consumes `sidx` (descgen metadata).
            nc.gpsimd.dma_scatter_add(
                out_ap=out[:, 0:8],  # 8-elem payload, 64-elem (256B) row stride
                in_ap=res[:, 0:8].rearrange("p (o e) -> p o e", o=1),
                idxs_ap=sidx[:],
                num_idxs=128,
                num_idxs_reg=128,
                elem_size=8,
                elem_step=64,
                prepare_only=True,
                sem=dma_sem,
            )

            # ---- gather: three span DMAs.  The HW SWDGE unroll consumes
            # exactly ONE index per destination partition row, so each DMA
            # fetches one contiguous 1029-element span (4 heat rows + 5)
            # per pair; the 5x5 window sits at static strides i*W+j inside.
            for t in range(TT):
                nc.gpsimd.indirect_dma_start(
                    out=blk[:, t * PITCH : (t + 1) * PITCH],
                    out_offset=None,
                    in_=heat[:, :],
                    in_offset=bass.IndirectOffsetOnAxis(ap=idxi[:, t : t + 1], axis=1),
                )

            # ---- masks (DVE, slack time during gather) -----------------
            pxc = pool.tile([PP, 2 * TT], F32)  # clip(round(x), 0, W-1)
            # op1=bypass reads idx16 without using it: an artificial edge so
            # the scheduler cannot slide the (slack-rich) mask chain into the
            # gaps of the critical index chain ahead of the idx cast.
            nc.vector.scalar_tensor_tensor(
                pxc[:],
                pxy[:],
                BIGF,
                _v(idxi[:], 0, [[1, TT]]).bitcast(F32).to_broadcast((PP, 2 * TT)) if False else _v(idxi[:], 0, [[0, 2], [1, TT]]).bitcast(F32),
                op0=A.subtract,
                op1=A.bypass,
            )
            nc.vector.tensor_scalar(pxc[:], pxc[:], 0.0, None, A.max)
            nc.vector.tensor_tensor(pxc[:], pxc[:], bnd1[:], A.min)
            ccp = pool.tile([PP, 2 * TT], F32)  # (cbase-px, ry0-py)
            nc.vector.tensor_tensor(ccp[:], basef[:], pxc[:], A.subtract)

            cm = pool.tile([PP, TT * RUNW], F32)  # col mask 0/1 [p, (t,j8)]
            nc.vector.tensor_tensor(
                cm[:], _v(ccp[:], 0, [[2, TT], [0, RUNW]]), jm[:], A.add
            )
            nc.vector.tensor_tensor(cm[:], cm[:], cm[:], A.mult)
            nc.vector.tensor_scalar(cm[:], cm[:], 4.5, None, A.is_le)

            rm = pool.tile([PP, TT * WN], F32)  # row mask 0/1 [p, (t,i)]
            nc.vector.tensor_tensor(
                rm[:], _v(ccp[:], 1, [[2, TT], [0, WN]]), ii[:], A.add
            )
            nc.vector.tensor_tensor(rm[:], rm[:], rm[:], A.mult)
            nc.vector.tensor_scalar(rm[:], rm[:], 4.5, None, A.is_le)

            mask = pool.tile([PP, TT * WN * RUNW], F32)  # 0/1 [p, (t,i,j8)]
            nc.vector.tensor_tensor(
                mask[:].rearrange("p (t i j) -> p t i j", i=WN, j=RUNW),
                _v(rm[:], 0, [[WN, TT], [1, WN], [0, RUNW]]),
                _v(cm[:], 0, [[RUNW, TT], [0, WN], [1, RUNW]]),
                A.mult,
            )
            # masked weight tables (slack time): mask*j and mask*i — lets
            # qx/qy multiply the RAW exp output without waiting for the
            # masked ez product
            mjj = pool.tile([PP, TT * WN * RUNW], F32)
            nc.vector.tensor_tensor(
                mjj[:].rearrange("p (t i j) -> p t i j", i=WN, j=RUNW),
                mask[:].rearrange("p (t i j) -> p t i j", i=WN, j=RUNW),
                _v(jj[:], 0, [[RUNW, TT], [0, WN], [1, RUNW]]),
                A.mult,
            )
            mii = pool.tile([PP, TT * WN * RUNW], F32)
            nc.vector.tensor_tensor(
                mii[:].rearrange("p (t i j) -> p t i j", i=WN, j=RUNW),
                mask[:].rearrange("p (t i j) -> p t i j", i=WN, j=RUNW),
                _v(ii[:], 0, [[WN, TT], [1, WN], [0, RUNW]]),
                A.mult,
            )

            # ---- post-gather ------------------------------------------
            # exp on the RAW gathered logits (|x| small, overflow-safe),
            # then kill masked entries with a 0/1 multiply: softmax weights
            # are identical and the pre-exp mask add leaves the critical
            # path (exp starts as soon as the gather lands).
            # big blocks: raw ez @0, qx @128, qy @256, ezm @384
            big = pool.tile([PP, 512], F32)
            red = pool.tile([PP, 9], F32)  # [(m, t)]: numx, numy, ssum
            nc.scalar.activation(
                _v(big[:], 0, [[WN * RUNW, TT], [RUNW, WN], [1, RUNW]]),
                _v(blk[:], 0, [[PITCH, TT], [W, WN], [1, RUNW]]),
                mybir.ActivationFunctionType.Exp,
            )
            # qy (Pool) and qx/ezm (DVE) all read the raw exp output — the
            # masks live in the precomputed tables, so all three products
            # start as soon as the activation lands
            nc.vector.tensor_tensor(
                big[:, 256:376], big[:, 0:120], mii[:], A.mult
            )
            nc.vector.tensor_tensor(
                big[:, 128:248], big[:, 0:120], mjj[:], A.mult
            )
            nc.vector.tensor_tensor(
                big[:, 384:504], big[:, 0:120], mask[:], A.mult
            )
            nc.vector.tensor_reduce(
                red[:].rearrange("p (m t) -> p m t", m=3),
                _v(big[:], 128, [[128, 3], [WN * RUNW, TT], [1, WN * RUNW]]),
                axis=mybir.AxisListType.X,
                op=A.add,
            )
            rinv = pool.tile([PP, TT], F32)
            nc.vector.reciprocal(rinv[:], red[:, 2 * TT : 3 * TT])
            nd = pool.tile([PP, 2 * TT], F32)  # [(c, t)] = num * rinv
            nc.vector.tensor_tensor(
                nd[:].rearrange("p (c t) -> p c t", c=2),
                red[:, 0 : 2 * TT].rearrange("p (c t) -> p c t", c=2),
                _v(rinv[:], 0, [[0, 2], [1, TT]]),
                A.mult,
            )
            nc.vector.tensor_tensor(
                res[:PP, 0 : 2 * TT].rearrange("p (c t) -> p c t", c=2),
                nd[:].rearrange("p (c t) -> p c t", c=2),
                _v(basef[:], 0, [[1, 2], [2, TT]]),  # [(c, t)] view of (t,c)
                A.add,
            )
            # res written (untracked tensor): raise res_sem from DVE, gate
            # the trigger on it manually.
            nc.vector.sem_inc(res_sem, 1)
            nc.gpsimd.wait_ge(res_sem, 1)
            nc.gpsimd.trigger_dma(count=None)

            if debug:
                nc.sync.dma_start(out=d_idx[:, :], in_=idx16[:])
                nc.sync.dma_start(out=d_blk[:, :], in_=blk[:])
                nc.sync.dma_start(out=d_mask[:, :], in_=mask[:])
                nc.sync.dma_start(out=d_red[:, :], in_=red[:])
                nc.sync.dma_start(out=d_base[:, :], in_=basef[:])
    nc.compile()

    # Tile books the prepared scatter's completion on a round-robin DMASW
    # lane sem, but a SWDGE descriptor carries exactly one completion sem —
    # the `sem=` we passed (store_dma_sem).  The lane sem is never
    # incremented, which deadlocks the postamble's drain wait.  Retarget any
    # orphaned DMASW wait (no matching updater anywhere in the program) to
    # the real completion sem; the drain semantics are identical.
    fn = nc.m.functions[0]

    # Drop the framework's unused const-AP memsets (f32-1.0 / bf16-1.0 /
    # u8-127) from the preamble: they serialize on Pool ahead of the entry
    # barrier and delay the first coords DMA.  Only const-float32-0.0 (the
    # activation bias) is actually read by this kernel.
    blk0 = fn.blocks[0]
    drop = []
    for i, inst in enumerate(blk0.instructions):
        if type(inst).__name__ == "InstMemset":
            s = str(inst.outs[0])
            if "const-float32-0.0" not in s and "const-" in s:
                drop.append(i)
    for i in reversed(drop):
        del blk0.instructions[i]

    updated_ids = set()
    for blk in fn.blocks:
        for inst in blk.instructions:
            si = inst.sync_info
            if si is not None:
                for u in si.on_update:
                    updated_ids.add(u.id)
    for blk in fn.blocks:
        for inst in blk.instructions:
            si = inst.sync_info
            if si is None:
                continue
            for w in si.on_wait:
                if (
                    w.ant_name
                    and w.ant_name.startswith("DMASW")
                    and w.id not in updated_ids
                ):
                    # neutralize the orphaned mid-postamble wait ...
                    w.wait_value = 0
    # ... and hold the kernel open on the store instead at the very end,
    # so the postamble's sem-clear chain overlaps the store's DMA latency.
    last_blk = fn.blocks[-1]
    for inst in reversed(last_blk.instructions):
        si = inst.sync_info
        if (
            isinstance(inst, mybir.InstEventSemaphore)
            and si is not None
            and len(si.on_wait) <= 1
            and inst.name.startswith("barrier_Pool")
        ):
            si.on_wait = list(si.on_wait) + [
                mybir.SyncWait(
                    sync_type="semaphore",
                    id=dma_sem.num,
                    ant_name="store_dma_sem",
                    wait_mode="sem-ge-imm",
                    wait_value=16,
                    wait_reg=None,
                )
            ]
            break
    return nc


_NC = None


def _get_nc():
    global _NC
    if _NC is None:
        _NC = build_program()
    return _NC


def make_in_maps(heatmaps: np.ndarray, coarse_coords: np.ndarray):
    heatmaps = np.ascontiguousarray(heatmaps, dtype=np.float32)
    coarse_coords = np.ascontiguousarray(coarse_coords, dtype=np.float32)
    # pair slot g = p + 91*t ; g in [0, 273), real pairs g < 272
    g = np.arange(SLOTS).reshape(TT, PP).T  # [p, t] -> g
    in_maps = []
    for m in range(NCORES):
        hs = np.zeros((R + 5, W), dtype=np.float32)
        hs[:R] = heatmaps[m * BS : (m + 1) * BS].reshape(R, W)
        cf = np.zeros((SLOTS, 2), dtype=np.float32)
        cf[:PAIRS] = coarse_coords[m * BS : (m + 1) * BS].reshape(PAIRS, 2)
        cs = cf[g].reshape(PP, TT * 2)  # [p, (t,c)]
        in_maps.append({"heat": hs, "coords": cs})
    return in_maps


def assemble_out(results) -> np.ndarray:
    outs = []
    for m in range(NCORES):
        o = results[m]["out"][:PP, : 2 * TT].reshape(PP, 2, TT)  # [p, c, t]
        flat = o.transpose(2, 0, 1).reshape(SLOTS, 2)[:PAIRS]  # g = p + 91*t
        outs.append(flat.reshape(BS, K, 2))
    return np.concatenate(outs, axis=0)


def kernel(heatmaps: np.ndarray, coarse_coords: np.ndarray) -> np.ndarray:
    nc = _get_nc()
    in_maps = make_in_maps(heatmaps, coarse_coords)
    results = run_bass_kernel_spmd(nc, in_maps, core_ids=list(range(NCORES)))
    return assemble_out(results.results)
